# revision 65
# baseline (speedup 1.0000x reference)
"""Trainium2 Bass kernel for the coverage-attention module.

Computation (per batch row b):
    dec_feature = dec_hidden @ W_dec.T + b_dec                    [DIM]
    att[s, d]   = enc_feature[s, d] + dec_feature[d] + coverage[s] * w_cov[d]
    score[s]    = sum_d v[d] * tanh(att[s, d])
    attn        = softmax(score) * enc_mask; attn /= sum(attn)
    context[d]  = sum_s attn[s] * enc_output[s, d]
    coverage_new = coverage + attn

Sharding: data-parallel over batch across 8 NeuronCores (4 rows each),
params replicated.  Host side only does layout (slicing / transposes).

On-chip layout: [s-tokens on partitions, d on free dim].  Per [128, 512]
half-tile the PE computes att = I@enc + [ones;cov].T@[dec_row;w_cov]
(identity pass-through + K=2 rank-1, all float32r) into PSUM, ACT applies
tanh, and one fused DVE scalar_tensor_tensor (x v with free-dim accum)
produces the score column.  Softmax runs entirely in the PE-transposed
[64, 128] layout with segmented per-batch sums via tiny block-diagonal
matmuls (no max-subtraction: |score| <= ||v||_1 ~ 26, far from fp32
overflow).  Pass 2 accumulates context with [128, 1] x [128, 512] PE
matmuls over enc_output.  The whole kernel is DMA-bound (~69 MB/core);
engines and buffer depths are arranged so the DMA stream never stalls.
"""

import os
import sys
from contextlib import ExitStack

import numpy as np

sys.path.insert(0, "/opt/trn_rl_repo")

import concourse.bacc as bacc
import concourse.bass as bass
import concourse.mybir as mybir
import concourse.tile as tile
from concourse import masks
from concourse.bass_utils import run_bass_kernel_spmd

B, S, DIM = 32, 2048, 1024
NCORES = 8
BL = B // NCORES          # 4 batch rows per core
P = 128                   # SBUF partitions
ST = S // P               # 16 s-tiles per batch row
NCOL = BL * ST            # 64 score columns per core
H = DIM // 2              # 512, psum bank half

F32 = mybir.dt.float32
F32R = mybir.dt.float32r
AX = mybir.AxisListType
OP = mybir.AluOpType
AF = mybir.ActivationFunctionType


def build_nc(phases=3):
    nc = bacc.Bacc("TRN2")

    # float32r inputs feed the PE only; bits are plain fp32.
    ef_d = nc.dram_tensor("enc_feature", [BL, S, DIM], F32R, kind="ExternalInput")
    eo_d = nc.dram_tensor("enc_output", [BL, S, DIM], F32R, kind="ExternalInput")
    mask_d = nc.dram_tensor("enc_mask", [BL, S], F32, kind="ExternalInput")
    cov_d = nc.dram_tensor("coverage", [BL, S], F32, kind="ExternalInput")
    # host-stacked [ones; coverage] rank-1 lhsT rows, one DMA per batch
    onescov_d = nc.dram_tensor("ones_cov", [BL, 2, S], F32R, kind="ExternalInput")
    # dec_hidden.T pre-packed on host into the exact SBUF tile layout
    # [128, 8*BL] (contiguous rows) — a strided load of the [DIM, BL] form
    # has 16 B innermost runs, which the DMA charges at 2x with a per-
    # descriptor floor (448 ns vs 91 ns for this packed form).
    dhp_d = nc.dram_tensor("dh_packed", [P, 8 * BL], F32R, kind="ExternalInput")
    wdT_d = nc.dram_tensor("w_decT", [DIM, DIM], F32R, kind="ExternalInput")
    bdec_d = nc.dram_tensor("b_dec", [1, DIM], F32R, kind="ExternalInput")
    v_d = nc.dram_tensor("v", [1, DIM], F32, kind="ExternalInput")
    # w_cov host-tiled x BL so rank-1 rhs row 1 fills with one DMA
    wcov4_d = nc.dram_tensor("w_cov4", [1, BL * DIM], F32R, kind="ExternalInput")

    ctx_d = nc.dram_tensor("context", [BL, DIM], F32, kind="ExternalOutput")
    attn_d = nc.dram_tensor("attn", [BL, S], F32, kind="ExternalOutput")
    covnew_d = nc.dram_tensor("coverage_new", [BL, S], F32, kind="ExternalOutput")


    with tile.TileContext(nc) as tc, ExitStack() as ctx:
        cp = ctx.enter_context(tc.tile_pool(name="const", bufs=1))
        rk = ctx.enter_context(tc.tile_pool(name="rank1", bufs=2))
        load1 = ctx.enter_context(tc.tile_pool(name="load1", bufs=12))
        thp = ctx.enter_context(tc.tile_pool(name="tanh", bufs=4))
        scrp = ctx.enter_context(tc.tile_pool(name="scr", bufs=1))
        ps1 = ctx.enter_context(
            tc.tile_pool(name="ps1", bufs=3, space=bass.MemorySpace.PSUM)
        )
        psctx = ctx.enter_context(
            tc.tile_pool(name="psctx", bufs=2, space=bass.MemorySpace.PSUM)
        )
        psmisc = ctx.enter_context(
            tc.tile_pool(name="psmisc", bufs=1, space=bass.MemorySpace.PSUM)
        )

        # ---------------- Phase 0: params & dec_feature ----------------

        dh_sb = cp.tile([P, 8 * BL], F32R, tag="dh")
        nc.sync.dma_start(dh_sb[:], dhp_d[:])
        wd_sb = cp.tile([P, 8 * DIM], F32R, tag="wd")
        for ki in range(8):
            nc.sync.dma_start(
                wd_sb[:, ki * DIM:(ki + 1) * DIM],
                wdT_d[ki * P:(ki + 1) * P, :],
            )

        v_sb = cp.tile([1, DIM], F32, tag="v")
        nc.sync.dma_start(v_sb[:], v_d[:])
        v_bc = cp.tile([P, DIM], F32, tag="vbc")
        nc.gpsimd.partition_broadcast(v_bc[:], v_sb[:])

        bdec_sb = cp.tile([1, DIM], F32R, tag="bdec")
        nc.sync.dma_start(bdec_sb[:], bdec_d[:])

        ident_t = cp.tile([P, P], F32, tag="identt")
        masks.make_identity(nc, ident_t[:])
        ident_r = cp.tile([P, P], F32R, tag="identr")
        nc.scalar.copy(ident_r[:], ident_t[:])

        # Block-diagonal ones [64, 4] (blk[p, m] = 1 iff p // 16 == m) and its
        # transpose, for segmented per-batch softmax sums / broadcasts via PE.
        blk = cp.tile([NCOL, BL], F32, tag="blk")
        nc.gpsimd.memset(blk[:], 1.0)
        nc.gpsimd.affine_select(
            out=blk[:], in_=blk[:], pattern=[[-ST, BL]],
            compare_op=OP.is_ge, fill=0.0, base=0, channel_multiplier=1,
        )
        nc.gpsimd.affine_select(
            out=blk[:], in_=blk[:], pattern=[[ST, BL]],
            compare_op=OP.is_ge, fill=0.0, base=ST - 1, channel_multiplier=-1,
        )
        blkT = cp.tile([BL, NCOL], F32, tag="blkT")
        nc.gpsimd.memset(blkT[:], 1.0)
        nc.gpsimd.affine_select(
            out=blkT[:], in_=blkT[:], pattern=[[1, NCOL]],
            compare_op=OP.is_ge, fill=0.0, base=0, channel_multiplier=-ST,
        )
        nc.gpsimd.affine_select(
            out=blkT[:], in_=blkT[:], pattern=[[-1, NCOL]],
            compare_op=OP.is_ge, fill=0.0, base=ST - 1, channel_multiplier=ST,
        )

        ones_f = cp.tile([1, H], F32, tag="onesf")
        nc.gpsimd.memset(ones_f[:], 1.0)
        ones = cp.tile([1, BL], F32R, tag="ones")
        nc.scalar.copy(ones[:], ones_f[:, 0:BL])

        # Combined rank-1 rhs [2, BL*DIM]: row0 = dec_feature rows (computed
        # with M=1 matmuls straight onto partition 0 — no cross-partition
        # move), row1 = w_cov per batch.  rhs for (b, h) = slice [2, H].
        rhs_all = cp.tile([2, BL * DIM], F32R, tag="rhsall")
        nc.sync.dma_start(rhs_all[1:2, :], wcov4_d[:])
        def emit_dec_chain(b):
            """dec_feature row b (+b_dec) -> rhs_all[0, b*DIM:(b+1)*DIM]."""
            for h in range(2):
                ps_dec = psmisc.tile([1, H], F32, tag="psdec", bufs=2,
                                     name=f"psdec{b}_{h}")
                for ki in range(8):
                    nc.tensor.matmul(
                        ps_dec[:],
                        dh_sb[:, ki * BL + b: ki * BL + b + 1],
                        wd_sb[:, ki * DIM + h * H: ki * DIM + (h + 1) * H],
                        start=(ki == 0),
                        stop=False,
                    )
                nc.tensor.matmul(
                    ps_dec[:],
                    ones[:, 0:1],
                    bdec_sb[:, h * H:(h + 1) * H],
                    start=False,
                    stop=True,
                )
                nc.scalar.copy(
                    rhs_all[0:1, b * DIM + h * H: b * DIM + (h + 1) * H],
                    ps_dec[:],
                )

        load2 = ctx.enter_context(tc.tile_pool(name="load2", bufs=10))

        if phases == 0:
            for b in range(BL):
                emit_dec_chain(b)
            nc.sync.dma_start(
                ctx_d[:].rearrange("b d -> (b d)").rearrange("(one n) -> one n", one=1),
                rhs_all[0:1, :].bitcast(F32),
            )

        # ---------------- Phase 1: scores ----------------
        score_st = cp.tile([P, NCOL], F32, tag="score")
        for b in range(BL if phases >= 1 else 0):
            # Per-batch rank-1 lhsT (rotating 2-deep pool):
            #   lhs_b [2, S]: row0 = ones, row1 = coverage[b] (host-stacked)
            lhs_b = rk.tile([2, S], F32R, tag="lhs", name=f"lhs{b}")
            nc.sync.dma_start(lhs_b[:], onescov_d[b, :, :])
            emit_dec_chain(b)
            for si in range(ST):
                col = b * ST + si
                ef = load1.tile([P, DIM], F32R, tag="ef", name="ef")
                nc.sync.dma_start(ef[:], ef_d[b, si * P:(si + 1) * P, :])
                th = thp.tile([P, DIM], F32, tag="th", name="th")
                for h in range(2):
                    ph = ps1.tile([P, H], F32, tag="ps1", name="ph")
                    nc.tensor.matmul(
                        ph[:], ident_r[:], ef[:, h * H:(h + 1) * H],
                        start=True, stop=False,
                    )
                    nc.tensor.matmul(
                        ph[:],
                        lhs_b[:, si * P:(si + 1) * P],
                        rhs_all[:, b * DIM + h * H: b * DIM + (h + 1) * H],
                        start=False, stop=True,
                    )
                    nc.scalar.activation(th[:, h * H:(h + 1) * H], ph[:], AF.Tanh)
                sc = scrp.tile([P, DIM], F32, tag="scr", name="sc")
                nc.vector.scalar_tensor_tensor(
                    out=sc[:],
                    in0=th[:],
                    scalar=1.0,
                    in1=v_bc[:],
                    op0=OP.mult,
                    op1=OP.mult,
                    accum_out=score_st[:, col:col + 1],
                )

        if phases == 1:
            nc.sync.dma_start(
                attn_d[:].rearrange("b (x c) -> (b x) c", c=NCOL), score_st[:]
            )

        # ---- Phase S: softmax entirely in the transposed [64, 128] layout ----
        # Rows = (b, si) blocks, columns = the 128 tokens of the block; batch b
        # owns partitions 16b..16b+15.  No max-subtraction: |score| <= ||v||_1
        # ~ 26 so exp stays far inside the fp32 range; the normalized softmax
        # matches the max-subtracted reference to fp32 rounding.
        if phases >= 2:
            mask_t = cp.tile([NCOL, P], F32, tag="maskT")
            nc.sync.dma_start(
                mask_t[:], mask_d[:].rearrange("b (si p) -> (b si) p", p=P)
            )
            cov_t = cp.tile([NCOL, P], F32, tag="covT")
            nc.sync.dma_start(
                cov_t[:], cov_d[:].rearrange("b (si p) -> (b si) p", p=P)
            )

            ps_t1 = psmisc.tile([NCOL, P], F32, tag="misc")
            nc.tensor.transpose(ps_t1[:], score_st[:], ident_t[:])
            exp_t = cp.tile([NCOL, P], F32, tag="expT")
            nc.scalar.activation(exp_t[:], ps_t1[:], AF.Exp)
            # masked numerator (in place)
            nc.vector.tensor_mul(exp_t[:], exp_t[:], mask_t[:])
            part = cp.tile([NCOL, 1], F32, tag="part")
            nc.vector.tensor_reduce(part[:], exp_t[:], axis=AX.X, op=OP.add)
            # per-batch denominators: blk.T @ part -> [BL, 1]
            ps_s4 = psmisc.tile([BL, 1], F32, tag="misc", name="ps_s4")
            nc.tensor.matmul(ps_s4[:], blk[:], part[:], start=True, stop=True)
            rinv = cp.tile([BL, 1], F32, tag="rinv")
            nc.vector.reciprocal(rinv[:], ps_s4[:])
            # broadcast 1/denominator back to the 64 rows: blkT.T @ rinv
            ps_r64 = psmisc.tile([NCOL, 1], F32, tag="misc", name="ps_r64")
            nc.tensor.matmul(ps_r64[:], blkT[:], rinv[:], start=True, stop=True)
            rinv64 = cp.tile([NCOL, 1], F32, tag="rinv64")
            nc.scalar.copy(rinv64[:], ps_r64[:])
            attn_t = cp.tile([NCOL, P], F32, tag="attnT")
            nc.vector.tensor_scalar_mul(attn_t[:], exp_t[:], rinv64[:, 0:1])
            # coverage_new (in place over cov_t)
            nc.vector.tensor_add(cov_t[:], attn_t[:], cov_t[:])
            attn_store = nc.gpsimd.dma_start(
                attn_d[:].rearrange("b (si p) -> (b si) p", p=P), attn_t[:]
            )
            covnew_store = nc.gpsimd.dma_start(
                covnew_d[:].rearrange("b (si p) -> (b si) p", p=P), cov_t[:]
            )

        if phases >= 3:
            # attn columns for pass 2: transpose [64, 128] -> [128, 64]
            ps_t2 = psmisc.tile([P, NCOL], F32, tag="misc", name="ps_t2")
            nc.tensor.transpose(ps_t2[:], attn_t[:], ident_t[0:NCOL, 0:NCOL])
            attn_cols = cp.tile([P, NCOL], F32R, tag="acols")
            nc.scalar.copy(attn_cols[:], ps_t2[:])

            # ---------------- Phase 2: context ----------------
            for b in range(BL):
                ctx_sb = cp.tile([1, DIM], F32, tag="ctxsb", bufs=2, name=f"ctx{b}")
                pc = [
                    psctx.tile([1, H], F32, tag="pctx", name=f"pctx{b}_0"),
                    psctx.tile([1, H], F32, tag="pctx", name=f"pctx{b}_1"),
                ]
                for si in range(ST):
                    col = b * ST + si
                    eo = load2.tile([P, DIM], F32R, tag="eo", name="eo")
                    eo_dma = nc.sync.dma_start(eo[:], eo_d[b, si * P:(si + 1) * P, :])
                    if b == BL - 1 and si == ST - 1:
                        last_eo_dma = eo_dma
                    for h in range(2):
                        nc.tensor.matmul(
                            pc[h][:],
                            attn_cols[:, col:col + 1],
                            eo[:, h * H:(h + 1) * H],
                            start=(si == 0),
                            stop=(si == ST - 1),
                        )
                for h in range(2):
                    nc.scalar.copy(ctx_sb[:, h * H:(h + 1) * H], pc[h][:])
                # SWDGE for mid-stream batches: a compute-dependent store
                # must not head-of-line block the enc_output load stream.
                # The last batch uses HWDGE (nothing queued behind it, and
                # the SWDGE descriptor latency would lengthen the tail).
                if b < BL - 1:
                    nc.gpsimd.dma_start(ctx_d[b:b + 1, :], ctx_sb[:])
                else:
                    nc.sync.dma_start(ctx_d[b:b + 1, :], ctx_sb[:])

            # Delay the attn/coverage stores until the load stream is done:
            # their transfers then ride the post-last-byte drain window
            # instead of consuming saturated mid-stream DMA time.
            from concourse.tile_rust import add_dep_helper
            add_dep_helper(attn_store.ins, last_eo_dma.ins, sync=True,
                           reason="store after load stream")
            add_dep_helper(covnew_store.ins, last_eo_dma.ins, sync=True,
                           reason="store after load stream")

    nc.compile()
    return nc


_NC = {}


def _get_nc(phases=3):
    if phases not in _NC:
        _NC[phases] = build_nc(phases)
    return _NC[phases]


def kernel(dec_hidden, enc_output, enc_feature, enc_mask, sec_attn, coverage,
           W_dec, b_dec, v, w_cov, _trace=False, _trace_kwargs=None):
    del sec_attn  # unused by the reference computation
    nc = _get_nc(3)

    f32 = lambda a: np.ascontiguousarray(np.asarray(a, dtype=np.float32))
    dec_hidden = f32(dec_hidden)
    enc_output = f32(enc_output)
    enc_feature = f32(enc_feature)
    enc_mask = f32(enc_mask)
    coverage = f32(coverage)
    wdT = f32(np.asarray(W_dec, dtype=np.float32).T)
    b_dec = f32(b_dec).reshape(1, DIM)
    v = f32(v).reshape(1, DIM)
    w_cov = f32(w_cov).reshape(1, DIM)

    in_maps = []
    for c in range(NCORES):
        sl = slice(c * BL, (c + 1) * BL)
        in_maps.append({
            "enc_feature": enc_feature[sl],
            "enc_output": enc_output[sl],
            "enc_mask": enc_mask[sl],
            "coverage": coverage[sl],
            "ones_cov": np.ascontiguousarray(
                np.stack([np.ones_like(coverage[sl]), coverage[sl]], axis=1)
            ),
            "dh_packed": f32(
                dec_hidden[sl].T.reshape(8, P, BL).transpose(1, 0, 2).reshape(P, 8 * BL)
            ),
            "w_decT": wdT,
            "b_dec": b_dec,
            "v": v,
            "w_cov4": np.ascontiguousarray(np.tile(w_cov, (1, BL))),
        })

    kwargs = {}
    if _trace:
        kwargs = dict(trace=True, trace_kwargs=_trace_kwargs or {})
    res = run_bass_kernel_spmd(nc, in_maps, core_ids=list(range(NCORES)), **kwargs)
    outs = res.results
    context = np.concatenate([outs[c]["context"] for c in range(NCORES)], axis=0)
    attn = np.concatenate([outs[c]["attn"] for c in range(NCORES)], axis=0)
    covnew = np.concatenate(
        [outs[c]["coverage_new"] for c in range(NCORES)], axis=0
    )
    if _trace:
        return (context, attn, covnew), res
    return context, attn, covnew
